# revision 14
# baseline (speedup 1.0000x reference)
"""Trainium2 Bass kernel for nn_CrossAttentionExpert.

Three single-query cross-attention "experts" (id/txt/vis), each attending over
the other two modalities (K=2 kv positions), outputs concatenated, fused by a
linear layer and LayerNorm.

Strategy (per spec sharding hint): pure data parallel over 8 NeuronCores,
batch 16384 -> 2048 rows/core. Weights replicated.

On-core dataflow: feature-major ("transposed") activations so every GEMM uses
natural-layout weight blocks as the stationary operand and activations as the
moving operand.  All GEMM operands are bf16 (fp32 PSUM accumulation).  Weights
are transposed+cast once on-chip into a bf16 DRAM scratch, then streamed per
pass.  The 2048 rows/core are processed in 4 passes of 512.

Attention math per expert (K=2) reduces to a sigmoid gate:
  w_a = sigmoid((s_a - s_b)/sqrt(D)),  o = V_b + w_a * (V_a - V_b)
Per-head score reduction and per-head gate broadcast are done with small
constant selector matmuls on the PE (partition-dim reductions/broadcasts).
"""

import sys

sys.path.insert(0, "/opt/trn_rl_repo")

import numpy as np

import concourse.bass as bass
import concourse.bacc as bacc
import concourse.mybir as mybir
import concourse.tile as tile
from concourse import bass_utils
from concourse.masks import make_identity

F32 = mybir.dt.float32
BF16 = mybir.dt.bfloat16
AF = mybir.ActivationFunctionType
ALU = mybir.AluOpType
AX = mybir.AxisListType

B, E, H, D = 16384, 1024, 16, 64
NCORES = 8
BC = B // NCORES          # 2048 rows per core
BSUB = 512                # rows per pass
NPASS = BC // BSUB        # 4
EC = E // 128             # 8 feature chunks
LN_EPS = 1e-5

CFG = {
    "mm_bufs": 4, "attn_bufs": 2, "tp_bufs": 2,
    "wband_bufs": 6, "fband_bufs": 2, "qkv_bufs": 3, "atp_bufs": 3,
    "xpool_bufs": 3, "ypool_bufs": 3, "small_bufs": 3, "prep_bufs": 2,
}

FEATS = ["id_feat", "txt_feat", "vis_feat"]
EXPERTS = [  # (name, q_idx, kv_a_idx, kv_b_idx)
    ("id", 0, 1, 2),
    ("txt", 1, 0, 2),
    ("vis", 2, 0, 1),
]


def _build_selectors(nc, sel, selt):
    """sel: [128, 8*16] bf16, chunk k cols k*16:(k+1)*16:
         sel_k[d, h] = 1 iff h == 2k + d//64   (score head-reduce, lhsT)
       selt: [16, 8*128] bf16, chunk k cols k*128:(k+1)*128:
         selt_k[h, d] = 1 iff h == 2k + d//64  (gate head-broadcast, lhsT)
    Condition h == 2k + d//64  <=>  -63 <= 64h - 128k - d <= 0.
    """
    nc.gpsimd.memset(sel, 1.0)
    nc.gpsimd.memset(selt, 1.0)
    for k in range(8):
        s = sel[:, k * 16:(k + 1) * 16]
        # keep where 128k + d - 64h >= 0
        nc.gpsimd.affine_select(out=s, in_=s, compare_op=ALU.is_ge, fill=0.0,
                                base=128 * k, pattern=[[-64, 16]],
                                channel_multiplier=1)
        # keep where 64h - 128k - d + 63 >= 0
        nc.gpsimd.affine_select(out=s, in_=s, compare_op=ALU.is_ge, fill=0.0,
                                base=63 - 128 * k, pattern=[[64, 16]],
                                channel_multiplier=-1)
        t = selt[:, k * 128:(k + 1) * 128]
        # keep where 128k + d - 64h >= 0   (partition=h, free=d)
        nc.gpsimd.affine_select(out=t, in_=t, compare_op=ALU.is_ge, fill=0.0,
                                base=128 * k, pattern=[[1, 128]],
                                channel_multiplier=-64)
        # keep where 64h - 128k - d + 63 >= 0
        nc.gpsimd.affine_select(out=t, in_=t, compare_op=ALU.is_ge, fill=0.0,
                                base=63 - 128 * k, pattern=[[-1, 128]],
                                channel_multiplier=64)


def _mm(nc, out, lhsT, rhs, start, stop):
    nc.tensor.ldweights(lhsT)
    nc.tensor.matmul(out, lhsT=lhsT, rhs=rhs, start=start, stop=stop)


def build_program(iters=1, passes=NPASS):
    nc = bacc.Bacc("TRN2", target_bir_lowering=False, debug=False,
                   num_devices=NCORES)

    feat_aps = [nc.dram_tensor(n, [BC, E], F32, kind="ExternalInput").ap()
                for n in FEATS]
    w_in, b_in, w_out, b_out = {}, {}, {}, {}
    for name, _, _, _ in EXPERTS:
        w_in[name] = nc.dram_tensor(f"w_in_{name}", [3 * E, E], F32,
                                    kind="ExternalInput").ap()
        b_in[name] = nc.dram_tensor(f"b_in_{name}", [3 * E], F32,
                                    kind="ExternalInput").ap()
        w_out[name] = nc.dram_tensor(f"w_out_{name}", [E, E], F32,
                                     kind="ExternalInput").ap()
        b_out[name] = nc.dram_tensor(f"b_out_{name}", [E], F32,
                                     kind="ExternalInput").ap()
    w_fuse = nc.dram_tensor("w_fuse", [E, 3 * E], F32, kind="ExternalInput").ap()
    b_fuse = nc.dram_tensor("b_fuse", [E], F32, kind="ExternalInput").ap()
    ln_g = nc.dram_tensor("ln_g", [E], F32, kind="ExternalInput").ap()
    ln_b = nc.dram_tensor("ln_b", [E], F32, kind="ExternalInput").ap()
    out_ap = nc.dram_tensor("out", [BC, E], F32, kind="ExternalOutput").ap()

    with tile.TileContext(nc) as tc:
        _emit(nc, tc, feat_aps, w_in, b_in, w_out, b_out, w_fuse, b_fuse,
              ln_g, ln_b, out_ap, iters, passes)
    nc.compile()
    return nc


def _emit(nc, tc, feat_aps, w_in, b_in, w_out, b_out, w_fuse, b_fuse,
          ln_g, ln_b, out_ap, iters=1, passes=NPASS):
    from contextlib import ExitStack
    ctx = ExitStack()
    with ctx:
        # ---------------- pools ----------------
        consts = ctx.enter_context(tc.tile_pool(name="consts", bufs=1))
        dram = ctx.enter_context(tc.tile_pool(name="dram", bufs=1, space="DRAM"))
        mm_ps = ctx.enter_context(tc.tile_pool(name="mm_ps", bufs=CFG["mm_bufs"], space="PSUM"))
        at_ps = ctx.enter_context(tc.tile_pool(name="at_ps", bufs=CFG["attn_bufs"], space="PSUM"))
        tp_ps = ctx.enter_context(tc.tile_pool(name="tp_ps", bufs=CFG["tp_bufs"], space="PSUM"))

        # ---------------- constants ----------------
        ident_b = consts.tile([128, 128], BF16, tag="ident_b")
        make_identity(nc, ident_b)
        ident_f = consts.tile([128, 128], F32, tag="ident_f")
        make_identity(nc, ident_f)
        sel = consts.tile([128, 8 * 16], BF16, tag="sel")
        selt = consts.tile([16, 8 * 128], BF16, tag="selt")
        _build_selectors(nc, sel, selt)

        bias_in_sb, bias_out_sb = {}, {}
        for name, _, _, _ in EXPERTS:
            t = consts.tile([128, 24], F32, tag=f"bin_{name}")
            nc.gpsimd.dma_start(t, b_in[name].rearrange("(c p) -> p c", p=128))
            bias_in_sb[name] = t
            t = consts.tile([128, 8], F32, tag=f"bout_{name}")
            nc.gpsimd.dma_start(t, b_out[name].rearrange("(c p) -> p c", p=128))
            bias_out_sb[name] = t
        bfuse_sb = consts.tile([128, 8], F32, tag="bfuse")
        nc.gpsimd.dma_start(bfuse_sb, b_fuse.rearrange("(c p) -> p c", p=128))

        def bcast128(src_ap):
            t = consts.tile([128, E], F32, tag=f"bc_{src_ap.tensor.name}")
            rep = bass.AP(tensor=src_ap.tensor, offset=src_ap.offset,
                          ap=[[0, 128]] + [list(p) for p in src_ap.ap])
            nc.gpsimd.dma_start(out=t, in_=rep)
            return t

        g_bc = bcast128(ln_g)
        b_bc = bcast128(ln_b)
        eps_sb = consts.tile([128, 1], F32, tag="eps")
        nc.vector.memset(eps_sb, LN_EPS)

        # ---------------- phase A: weight transpose+cast to bf16 scratch ----
        def prep_weight(prep, w_ap, R, C, wtag):
            """w [R, C] fp32 DRAM -> returns wT [C, R] bf16 DRAM tile."""
            wT = dram.tile([C, R], BF16, tag=wtag, name=wtag)
            w_r = w_ap.rearrange("(ro p) c -> p ro c", p=128)  # [128, R/128, C]
            RO = R // 128
            for cs in range(C // 128):
                colf = prep.tile([128, RO, 128], F32, tag="colf")
                nc.sync.dma_start(colf, w_r[:, :, cs * 128:(cs + 1) * 128])
                colb = prep.tile([128, RO, 128], BF16, tag="colb")
                nc.any.tensor_copy(out=colb, in_=colf)
                wTrow = prep.tile([128, RO, 128], BF16, tag="wTrow")
                for ro in range(RO):
                    ps = tp_ps.tile([128, 128], BF16, tag="tp")
                    nc.tensor.transpose(ps, colb[:, ro, :], ident_b)
                    nc.any.tensor_copy(out=wTrow[:, ro, :], in_=ps)
                nc.sync.dma_start(
                    wT[cs * 128:(cs + 1) * 128, :].rearrange(
                        "p (a b) -> p a b", b=128),
                    wTrow)
            return wT

        w_inT, w_outT = {}, {}
        with tc.tile_pool(name="prep", bufs=CFG["prep_bufs"]) as prep:
            for name, _, _, _ in EXPERTS:
                w_inT[name] = prep_weight(prep, w_in[name], 3 * E, E,
                                          f"wTin_{name}")
                w_outT[name] = prep_weight(prep, w_out[name], E, E,
                                           f"wTout_{name}")
            w_fuseT = prep_weight(prep, w_fuse, E, 3 * E, "wTfuse")

        # ---------------- phase B pools ----------------
        xpool = ctx.enter_context(tc.tile_pool(name="xpool", bufs=CFG["xpool_bufs"]))
        xtp = ctx.enter_context(tc.tile_pool(name="xtp", bufs=1))
        qkv = ctx.enter_context(tc.tile_pool(name="qkv", bufs=CFG["qkv_bufs"]))
        qp = ctx.enter_context(tc.tile_pool(name="qp", bufs=1))
        atp = ctx.enter_context(tc.tile_pool(name="atp", bufs=CFG["atp_bufs"]))
        ytp = ctx.enter_context(tc.tile_pool(name="ytp", bufs=1))
        ypool = ctx.enter_context(tc.tile_pool(name="ypool", bufs=CFG["ypool_bufs"]))
        ysqp = ctx.enter_context(tc.tile_pool(name="ysqp", bufs=1))
        small = ctx.enter_context(tc.tile_pool(name="small", bufs=CFG["small_bufs"]))
        scp = ctx.enter_context(tc.tile_pool(name="scp", bufs=2))
        wband = ctx.enter_context(tc.tile_pool(name="wband", bufs=CFG["wband_bufs"]))
        fband = ctx.enter_context(tc.tile_pool(name="fband", bufs=CFG["fband_bufs"]))
        stats = ctx.enter_context(tc.tile_pool(name="stats", bufs=4))

        # rearranged views for band loads: [128, K/128, COLS]
        w_inT_r = {n: w_inT[n].rearrange("(ko p) e -> p ko e", p=128)
                   for n in w_inT}
        w_outT_r = {n: w_outT[n].rearrange("(ko p) e -> p ko e", p=128)
                    for n in w_outT}
        w_fuseT_r = w_fuseT.rearrange("(ko p) e -> p ko e", p=128)

        # ---------------- phase B ----------------
        def proj(wT_r, col_off, srcs, bias_sb, bias_col, outs, out_dtype):
            """outs[si][:, c, :] = (wT[:, col_off+c*128:...]^T @ srcs[si]) + bias
            srcs: list of [128, EC, BSUB] bf16 tiles; outs alloc'd by caller."""
            for c in range(EC):
                wb = wband.tile([128, EC, 128], BF16, tag="wb")
                nc.sync.dma_start(
                    wb, wT_r[:, :, col_off + c * 128:col_off + (c + 1) * 128])
                psums = [mm_ps.tile([128, BSUB], F32, tag="mm",
                                    name=f"mm_{si}")
                         for si in range(len(srcs))]
                for k in range(EC):
                    for si, src in enumerate(srcs):
                        _mm(nc, psums[si], wb[:, k, :], src[:, k, :],
                            (k == 0), (k == EC - 1))
                for si in range(len(srcs)):
                    nc.scalar.add(outs[si][:, c, :], psums[si],
                                  bias_sb[:, bias_col + c:bias_col + c + 1])

        def phase_b(_it=None):
          for p in range(passes):
            row0 = p * BSUB
            # -- transposed bf16 inputs XT[m]: [128, EC, BSUB]
            XT = []
            for m in range(3):
                xt = xtp.tile([128, EC, BSUB], BF16, tag=f"xt{m}")
                for bt in range(BSUB // 128):
                    xn = xpool.tile([128, E], F32, tag="xn")
                    nc.sync.dma_start(
                        xn, feat_aps[m][row0 + bt * 128:row0 + (bt + 1) * 128, :])
                    xb = xpool.tile([128, E], BF16, tag="xb")
                    nc.vector.tensor_copy(out=xb, in_=xn)
                    for eb in range(EC):
                        ps = tp_ps.tile([128, 128], BF16, tag="tp")
                        nc.tensor.transpose(ps, xb[:, eb * 128:(eb + 1) * 128],
                                            ident_b)
                        nc.vector.tensor_copy(
                            out=xt[:, eb, bt * 128:(bt + 1) * 128], in_=ps)
                XT.append(xt)

            AT = []
            for name, qi, ai, bi in EXPERTS:
                # -- projections
                QT = qp.tile([128, EC, BSUB], BF16, tag="qt")
                proj(w_inT_r[name], 0, [XT[qi]], bias_in_sb[name], 0,
                     [QT], BF16)
                KaT = qkv.tile([128, EC, BSUB], BF16, tag="kv")
                KbT = qkv.tile([128, EC, BSUB], BF16, tag="kv")
                proj(w_inT_r[name], E, [XT[ai], XT[bi]], bias_in_sb[name], 8,
                     [KaT, KbT], BF16)

                # -- scores + gate: wa = sigmoid((sa - sb)/8)
                sa = scp.tile([16, BSUB], F32, tag="sa")
                wa = small.tile([16, BSUB], BF16, tag="wa")
                for j, KT in enumerate((KaT, KbT)):
                    ps = at_ps.tile([128, BSUB], F32, tag="attn")
                    for k in range(EC):
                        mult = small.tile([128, BSUB], BF16, tag="mult")
                        nc.vector.tensor_mul(out=mult, in0=QT[:, k, :],
                                             in1=KT[:, k, :])
                        _mm(nc, ps[:16, :], sel[:, k * 16:(k + 1) * 16],
                            mult, (k == 0), (k == EC - 1))
                    if j == 0:
                        nc.vector.tensor_copy(out=sa, in_=ps[:16, :])
                    else:
                        dsc = scp.tile([16, BSUB], F32, tag="dsc")
                        nc.vector.tensor_sub(dsc, sa, ps[:16, :])
                        nc.scalar.activation(wa, dsc, AF.Sigmoid, scale=0.125)

                VaT = qkv.tile([128, EC, BSUB], BF16, tag="kv")
                VbT = qkv.tile([128, EC, BSUB], BF16, tag="kv")
                proj(w_inT_r[name], 2 * E, [XT[ai], XT[bi]], bias_in_sb[name],
                     16, [VaT, VbT], BF16)

                # -- combine: o = Vb + wa_exp * (Va - Vb)   (into VbT)
                for k in range(EC):
                    pse = at_ps.tile([128, BSUB], F32, tag="attn")
                    _mm(nc, pse, selt[:, k * 128:(k + 1) * 128], wa,
                        True, True)
                    diff = small.tile([128, BSUB], BF16, tag="diff")
                    nc.vector.tensor_sub(diff, VaT[:, k, :], VbT[:, k, :])
                    nc.vector.tensor_mul(out=diff, in0=pse, in1=diff)
                    nc.vector.tensor_add(VbT[:, k, :], diff, VbT[:, k, :])

                # -- output projection
                At = atp.tile([128, EC, BSUB], BF16, tag="at")
                proj(w_outT_r[name], 0, [VbT], bias_out_sb[name], 0,
                     [At], BF16)
                AT.append(At)

            # -- fuse: y^T = w_fuse^T-blocks @ concat(AT)
            YT = ytp.tile([128, EC, BSUB], F32, tag="yt")
            for c in range(EC):
                fb = fband.tile([128, 3 * EC, 128], BF16, tag="fb")
                nc.sync.dma_start(
                    fb, w_fuseT_r[:, :, c * 128:(c + 1) * 128])
                ps = mm_ps.tile([128, BSUB], F32, tag="mm")
                for k in range(3 * EC):
                    _mm(nc, ps, fb[:, k, :], AT[k // EC][:, k % EC, :],
                        (k == 0), (k == 3 * EC - 1))
                nc.scalar.add(YT[:, c, :], ps, bfuse_sb[:, c:c + 1])

            # -- transpose back + LayerNorm + store
            for bt in range(BSUB // 128):
                y = ypool.tile([128, E], F32, tag="y")
                for c in range(EC):
                    ps = tp_ps.tile([128, 128], F32, tag="tp")
                    nc.tensor.transpose(
                        ps, YT[:, c, bt * 128:(bt + 1) * 128], ident_f)
                    nc.scalar.copy(y[:, c * 128:(c + 1) * 128], ps)
                ssum = stats.tile([128, 1], F32, tag="ssum")
                nc.vector.reduce_sum(ssum, y, axis=AX.X)
                ysq = ysqp.tile([128, E], F32, tag="ysq")
                ss = stats.tile([128, 1], F32, tag="ss")
                nc.scalar.activation(ysq, y, AF.Square, accum_out=ss)
                mu = stats.tile([128, 1], F32, tag="mu")
                nc.vector.tensor_scalar_mul(mu, ssum, 1.0 / E)
                ex2 = stats.tile([128, 1], F32, tag="ex2")
                nc.vector.tensor_scalar_mul(ex2, ss, 1.0 / E)
                m2 = stats.tile([128, 1], F32, tag="m2")
                nc.vector.tensor_mul(out=m2, in0=mu, in1=mu)
                var = stats.tile([128, 1], F32, tag="var")
                nc.vector.tensor_sub(var, ex2, m2)
                std = stats.tile([128, 1], F32, tag="std")
                nc.scalar.activation(std, var, AF.Sqrt, bias=eps_sb)
                rstd = stats.tile([128, 1], F32, tag="rstd")
                nc.vector.reciprocal(rstd, std)
                nc.vector.tensor_scalar(y, y, mu, rstd, ALU.subtract, ALU.mult)
                nc.vector.tensor_mul(out=y, in0=y, in1=g_bc)
                nc.vector.tensor_add(y, y, b_bc)
                nc.sync.dma_start(
                    out_ap[row0 + bt * 128:row0 + (bt + 1) * 128, :], y)

        if iters == 1:
            phase_b()
        else:
            with tc.For_i(0, iters, 1) as _i:
                phase_b(_i)


_NC_CACHE = {}


def _get_program():
    if "nc" not in _NC_CACHE:
        _NC_CACHE["nc"] = build_program()
    return _NC_CACHE["nc"]


def _get_runner():
    """Cached jitted SPMD runner. Feats/outputs sharded over cores, weights
    replicated (sent once, not 8x)."""
    if "runner" in _NC_CACHE:
        return _NC_CACHE["runner"]
    import jax
    from jax.sharding import Mesh, PartitionSpec as P
    from jax.experimental.shard_map import shard_map
    from concourse import bass2jax
    from concourse.bass2jax import (_bass_exec_p, install_neuronx_cc_hook,
                                    partition_id_tensor)

    nc = _get_program()
    install_neuronx_cc_hook()
    assert nc.dbg_addr is None
    pid_name = (nc.partition_id_tensor.name
                if nc.partition_id_tensor is not None else None)

    in_names, out_names, out_avals = [], [], []
    for alloc in nc.m.functions[0].allocations:
        if not isinstance(alloc, mybir.MemoryLocationSet):
            continue
        name = alloc.memorylocations[0].name
        if alloc.kind == "ExternalInput":
            if name != pid_name:
                in_names.append(name)
        elif alloc.kind == "ExternalOutput":
            out_names.append(name)
            out_avals.append(jax.core.ShapedArray(
                tuple(alloc.tensor_shape), mybir.dt.np(alloc.dtype)))
    n_params = len(in_names)

    all_in_names = in_names + out_names + ([pid_name] if pid_name else [])

    def _body(*args):
        operands = list(args)
        if pid_name is not None:
            operands.append(partition_id_tensor())
        outs = _bass_exec_p.bind(
            *operands,
            out_avals=tuple(out_avals),
            in_names=tuple(all_in_names),
            out_names=tuple(out_names),
            lowering_input_output_aliases=(),
            sim_require_finite=True,
            sim_require_nnan=True,
            nc=nc,
        )
        return tuple(outs)

    devices = jax.devices()[:NCORES]
    mesh = Mesh(np.asarray(devices), ("core",))
    in_specs = tuple(P("core") if n in FEATS else P() for n in in_names) + \
        (P("core"),) * len(out_names)
    out_specs = (P("core"),) * len(out_names)
    sharded = jax.jit(
        shard_map(_body, mesh=mesh, in_specs=in_specs, out_specs=out_specs,
                  check_rep=False),
        donate_argnums=tuple(range(n_params, n_params + len(out_names))),
        keep_unused=True)
    _NC_CACHE["runner"] = (sharded, in_names, out_names, out_avals)
    return _NC_CACHE["runner"]


def kernel(**inputs):
    inputs = {k: np.asarray(v) for k, v in inputs.items()}
    sharded, in_names, out_names, out_avals = _get_runner()
    args = [inputs[n] for n in in_names]
    zeros = [np.zeros((NCORES * a.shape[0], *a.shape[1:]), a.dtype)
             for a in out_avals]
    outs = sharded(*args, *zeros)
    return np.asarray(outs[0])


# revision 15
# speedup vs baseline: 1.0238x; 1.0238x over previous
"""Trainium2 Bass kernel for nn_CrossAttentionExpert.

Three single-query cross-attention "experts" (id/txt/vis), each attending over
the other two modalities (K=2 kv positions), outputs concatenated, fused by a
linear layer and LayerNorm.

Strategy (per spec sharding hint): pure data parallel over 8 NeuronCores,
batch 16384 -> 2048 rows/core. Weights replicated.

On-core dataflow: feature-major ("transposed") activations so every GEMM uses
natural-layout weight blocks as the stationary operand and activations as the
moving operand.  All GEMM operands are bf16 (fp32 PSUM accumulation).  Weights
are transposed+cast once on-chip into a bf16 DRAM scratch, then streamed per
pass.  The 2048 rows/core are processed in 4 passes of 512.

Attention math per expert (K=2) reduces to a sigmoid gate:
  w_a = sigmoid((s_a - s_b)/sqrt(D)),  o = V_b + w_a * (V_a - V_b)
Per-head score reduction and per-head gate broadcast are done with small
constant selector matmuls on the PE (partition-dim reductions/broadcasts).
"""

import sys

sys.path.insert(0, "/opt/trn_rl_repo")

import numpy as np

import concourse.bass as bass
import concourse.bacc as bacc
import concourse.mybir as mybir
import concourse.tile as tile
from concourse import bass_utils
from concourse.masks import make_identity

F32 = mybir.dt.float32
BF16 = mybir.dt.bfloat16
AF = mybir.ActivationFunctionType
ALU = mybir.AluOpType
AX = mybir.AxisListType

B, E, H, D = 16384, 1024, 16, 64
NCORES = 8
BC = B // NCORES          # 2048 rows per core
BSUB = 512                # rows per pass
NPASS = BC // BSUB        # 4
EC = E // 128             # 8 feature chunks
LN_EPS = 1e-5

CFG = {
    "mm_bufs": 4, "attn_bufs": 2, "tp_bufs": 2,
    "wband_bufs": 6, "fband_bufs": 2, "qkv_bufs": 3, "atp_bufs": 3,
    "xpool_bufs": 3, "ypool_bufs": 3, "small_bufs": 3, "prep_bufs": 2,
}

FEATS = ["id_feat", "txt_feat", "vis_feat"]
EXPERTS = [  # (name, q_idx, kv_a_idx, kv_b_idx)
    ("id", 0, 1, 2),
    ("txt", 1, 0, 2),
    ("vis", 2, 0, 1),
]


def _build_selectors(nc, sel, selt):
    """sel: [128, 8*16] bf16, chunk k cols k*16:(k+1)*16:
         sel_k[d, h] = 1 iff h == 2k + d//64   (score head-reduce, lhsT)
       selt: [16, 8*128] bf16, chunk k cols k*128:(k+1)*128:
         selt_k[h, d] = 1 iff h == 2k + d//64  (gate head-broadcast, lhsT)
    Condition h == 2k + d//64  <=>  -63 <= 64h - 128k - d <= 0.
    """
    nc.gpsimd.memset(sel, 1.0)
    nc.gpsimd.memset(selt, 1.0)
    for k in range(8):
        s = sel[:, k * 16:(k + 1) * 16]
        # keep where 128k + d - 64h >= 0
        nc.gpsimd.affine_select(out=s, in_=s, compare_op=ALU.is_ge, fill=0.0,
                                base=128 * k, pattern=[[-64, 16]],
                                channel_multiplier=1)
        # keep where 64h - 128k - d + 63 >= 0
        nc.gpsimd.affine_select(out=s, in_=s, compare_op=ALU.is_ge, fill=0.0,
                                base=63 - 128 * k, pattern=[[64, 16]],
                                channel_multiplier=-1)
        t = selt[:, k * 128:(k + 1) * 128]
        # keep where 128k + d - 64h >= 0   (partition=h, free=d)
        nc.gpsimd.affine_select(out=t, in_=t, compare_op=ALU.is_ge, fill=0.0,
                                base=128 * k, pattern=[[1, 128]],
                                channel_multiplier=-64)
        # keep where 64h - 128k - d + 63 >= 0
        nc.gpsimd.affine_select(out=t, in_=t, compare_op=ALU.is_ge, fill=0.0,
                                base=63 - 128 * k, pattern=[[-1, 128]],
                                channel_multiplier=64)


def _mm(nc, out, lhsT, rhs, start, stop):
    nc.tensor.matmul(out, lhsT=lhsT, rhs=rhs, start=start, stop=stop)


def build_program(iters=1, passes=NPASS):
    nc = bacc.Bacc("TRN2", target_bir_lowering=False, debug=False,
                   num_devices=NCORES)

    feat_aps = [nc.dram_tensor(n, [BC, E], F32, kind="ExternalInput").ap()
                for n in FEATS]
    w_in, b_in, w_out, b_out = {}, {}, {}, {}
    for name, _, _, _ in EXPERTS:
        w_in[name] = nc.dram_tensor(f"w_in_{name}", [3 * E, E], F32,
                                    kind="ExternalInput").ap()
        b_in[name] = nc.dram_tensor(f"b_in_{name}", [3 * E], F32,
                                    kind="ExternalInput").ap()
        w_out[name] = nc.dram_tensor(f"w_out_{name}", [E, E], F32,
                                     kind="ExternalInput").ap()
        b_out[name] = nc.dram_tensor(f"b_out_{name}", [E], F32,
                                     kind="ExternalInput").ap()
    w_fuse = nc.dram_tensor("w_fuse", [E, 3 * E], F32, kind="ExternalInput").ap()
    b_fuse = nc.dram_tensor("b_fuse", [E], F32, kind="ExternalInput").ap()
    ln_g = nc.dram_tensor("ln_g", [E], F32, kind="ExternalInput").ap()
    ln_b = nc.dram_tensor("ln_b", [E], F32, kind="ExternalInput").ap()
    out_ap = nc.dram_tensor("out", [BC, E], F32, kind="ExternalOutput").ap()

    with tile.TileContext(nc) as tc:
        _emit(nc, tc, feat_aps, w_in, b_in, w_out, b_out, w_fuse, b_fuse,
              ln_g, ln_b, out_ap, iters, passes)
    nc.compile()
    return nc


def _emit(nc, tc, feat_aps, w_in, b_in, w_out, b_out, w_fuse, b_fuse,
          ln_g, ln_b, out_ap, iters=1, passes=NPASS):
    from contextlib import ExitStack
    ctx = ExitStack()
    with ctx:
        # ---------------- pools ----------------
        consts = ctx.enter_context(tc.tile_pool(name="consts", bufs=1))
        dram = ctx.enter_context(tc.tile_pool(name="dram", bufs=1, space="DRAM"))
        mm_ps = ctx.enter_context(tc.tile_pool(name="mm_ps", bufs=CFG["mm_bufs"], space="PSUM"))
        at_ps = ctx.enter_context(tc.tile_pool(name="at_ps", bufs=CFG["attn_bufs"], space="PSUM"))
        tp_ps = ctx.enter_context(tc.tile_pool(name="tp_ps", bufs=CFG["tp_bufs"], space="PSUM"))

        # ---------------- constants ----------------
        ident_b = consts.tile([128, 128], BF16, tag="ident_b")
        make_identity(nc, ident_b)
        ident_f = consts.tile([128, 128], F32, tag="ident_f")
        make_identity(nc, ident_f)
        sel = consts.tile([128, 8 * 16], BF16, tag="sel")
        selt = consts.tile([16, 8 * 128], BF16, tag="selt")
        _build_selectors(nc, sel, selt)

        bias_in_sb, bias_out_sb = {}, {}
        for name, _, _, _ in EXPERTS:
            t = consts.tile([128, 24], F32, tag=f"bin_{name}")
            nc.gpsimd.dma_start(t, b_in[name].rearrange("(c p) -> p c", p=128))
            bias_in_sb[name] = t
            t = consts.tile([128, 8], F32, tag=f"bout_{name}")
            nc.gpsimd.dma_start(t, b_out[name].rearrange("(c p) -> p c", p=128))
            bias_out_sb[name] = t
        bfuse_sb = consts.tile([128, 8], F32, tag="bfuse")
        nc.gpsimd.dma_start(bfuse_sb, b_fuse.rearrange("(c p) -> p c", p=128))

        def bcast128(src_ap):
            t = consts.tile([128, E], F32, tag=f"bc_{src_ap.tensor.name}")
            rep = bass.AP(tensor=src_ap.tensor, offset=src_ap.offset,
                          ap=[[0, 128]] + [list(p) for p in src_ap.ap])
            nc.gpsimd.dma_start(out=t, in_=rep)
            return t

        g_bc = bcast128(ln_g)
        b_bc = bcast128(ln_b)
        eps_sb = consts.tile([128, 1], F32, tag="eps")
        nc.vector.memset(eps_sb, LN_EPS)

        # ---------------- phase A: weight transpose+cast to bf16 scratch ----
        def prep_weight(prep, w_ap, R, C, wtag):
            """w [R, C] fp32 DRAM -> returns wT [C, R] bf16 DRAM tile."""
            wT = dram.tile([C, R], BF16, tag=wtag, name=wtag)
            w_r = w_ap.rearrange("(ro p) c -> p ro c", p=128)  # [128, R/128, C]
            RO = R // 128
            for cs in range(C // 128):
                colf = prep.tile([128, RO, 128], F32, tag="colf")
                nc.sync.dma_start(colf, w_r[:, :, cs * 128:(cs + 1) * 128])
                colb = prep.tile([128, RO, 128], BF16, tag="colb")
                nc.any.tensor_copy(out=colb, in_=colf)
                wTrow = prep.tile([128, RO, 128], BF16, tag="wTrow")
                for ro in range(RO):
                    ps = tp_ps.tile([128, 128], BF16, tag="tp")
                    nc.tensor.transpose(ps, colb[:, ro, :], ident_b)
                    nc.any.tensor_copy(out=wTrow[:, ro, :], in_=ps)
                nc.sync.dma_start(
                    wT[cs * 128:(cs + 1) * 128, :].rearrange(
                        "p (a b) -> p a b", b=128),
                    wTrow)
            return wT

        w_inT, w_outT = {}, {}
        with tc.tile_pool(name="prep", bufs=CFG["prep_bufs"]) as prep:
            for name, _, _, _ in EXPERTS:
                w_inT[name] = prep_weight(prep, w_in[name], 3 * E, E,
                                          f"wTin_{name}")
                w_outT[name] = prep_weight(prep, w_out[name], E, E,
                                           f"wTout_{name}")
            w_fuseT = prep_weight(prep, w_fuse, E, 3 * E, "wTfuse")

        # ---------------- phase B pools ----------------
        xpool = ctx.enter_context(tc.tile_pool(name="xpool", bufs=CFG["xpool_bufs"]))
        xtp = ctx.enter_context(tc.tile_pool(name="xtp", bufs=1))
        qkv = ctx.enter_context(tc.tile_pool(name="qkv", bufs=CFG["qkv_bufs"]))
        qp = ctx.enter_context(tc.tile_pool(name="qp", bufs=1))
        atp = ctx.enter_context(tc.tile_pool(name="atp", bufs=CFG["atp_bufs"]))
        ytp = ctx.enter_context(tc.tile_pool(name="ytp", bufs=1))
        ypool = ctx.enter_context(tc.tile_pool(name="ypool", bufs=CFG["ypool_bufs"]))
        ysqp = ctx.enter_context(tc.tile_pool(name="ysqp", bufs=1))
        small = ctx.enter_context(tc.tile_pool(name="small", bufs=CFG["small_bufs"]))
        scp = ctx.enter_context(tc.tile_pool(name="scp", bufs=2))
        wband = ctx.enter_context(tc.tile_pool(name="wband", bufs=CFG["wband_bufs"]))
        fband = ctx.enter_context(tc.tile_pool(name="fband", bufs=CFG["fband_bufs"]))
        stats = ctx.enter_context(tc.tile_pool(name="stats", bufs=4))

        # rearranged views for band loads: [128, K/128, COLS]
        w_inT_r = {n: w_inT[n].rearrange("(ko p) e -> p ko e", p=128)
                   for n in w_inT}
        w_outT_r = {n: w_outT[n].rearrange("(ko p) e -> p ko e", p=128)
                    for n in w_outT}
        w_fuseT_r = w_fuseT.rearrange("(ko p) e -> p ko e", p=128)

        # ---------------- phase B ----------------
        def proj(wT_r, col_off, srcs, bias_sb, bias_col, outs, out_dtype):
            """outs[si][:, c, :] = (wT[:, col_off+c*128:...]^T @ srcs[si]) + bias
            srcs: list of [128, EC, BSUB] bf16 tiles; outs alloc'd by caller."""
            for c in range(EC):
                wb = wband.tile([128, EC, 128], BF16, tag="wb")
                nc.sync.dma_start(
                    wb, wT_r[:, :, col_off + c * 128:col_off + (c + 1) * 128])
                psums = [mm_ps.tile([128, BSUB], F32, tag="mm",
                                    name=f"mm_{si}")
                         for si in range(len(srcs))]
                for si, src in enumerate(srcs):
                    for k in range(EC):
                        _mm(nc, psums[si], wb[:, k, :], src[:, k, :],
                            (k == 0), (k == EC - 1))
                for si in range(len(srcs)):
                    nc.scalar.add(outs[si][:, c, :], psums[si],
                                  bias_sb[:, bias_col + c:bias_col + c + 1])

        def phase_b(_it=None):
          for p in range(passes):
            row0 = p * BSUB
            # -- transposed bf16 inputs XT[m]: [128, EC, BSUB]
            XT = []
            for m in range(3):
                xt = xtp.tile([128, EC, BSUB], BF16, tag=f"xt{m}")
                for bt in range(BSUB // 128):
                    xn = xpool.tile([128, E], F32, tag="xn")
                    nc.sync.dma_start(
                        xn, feat_aps[m][row0 + bt * 128:row0 + (bt + 1) * 128, :])
                    xb = xpool.tile([128, E], BF16, tag="xb")
                    nc.vector.tensor_copy(out=xb, in_=xn)
                    for eb in range(EC):
                        ps = tp_ps.tile([128, 128], BF16, tag="tp")
                        nc.tensor.transpose(ps, xb[:, eb * 128:(eb + 1) * 128],
                                            ident_b)
                        nc.vector.tensor_copy(
                            out=xt[:, eb, bt * 128:(bt + 1) * 128], in_=ps)
                XT.append(xt)

            AT = []
            for name, qi, ai, bi in EXPERTS:
                # -- projections
                QT = qp.tile([128, EC, BSUB], BF16, tag="qt")
                proj(w_inT_r[name], 0, [XT[qi]], bias_in_sb[name], 0,
                     [QT], BF16)
                KaT = qkv.tile([128, EC, BSUB], BF16, tag="kv")
                KbT = qkv.tile([128, EC, BSUB], BF16, tag="kv")
                proj(w_inT_r[name], E, [XT[ai], XT[bi]], bias_in_sb[name], 8,
                     [KaT, KbT], BF16)

                # -- scores + gate: wa = sigmoid((sa - sb)/8)
                sa = scp.tile([16, BSUB], F32, tag="sa")
                wa = small.tile([16, BSUB], BF16, tag="wa")
                for j, KT in enumerate((KaT, KbT)):
                    ps = at_ps.tile([128, BSUB], F32, tag="attn")
                    for k in range(EC):
                        mult = small.tile([128, BSUB], BF16, tag="mult")
                        nc.vector.tensor_mul(out=mult, in0=QT[:, k, :],
                                             in1=KT[:, k, :])
                        _mm(nc, ps[:16, :], sel[:, k * 16:(k + 1) * 16],
                            mult, (k == 0), (k == EC - 1))
                    if j == 0:
                        nc.vector.tensor_copy(out=sa, in_=ps[:16, :])
                    else:
                        dsc = scp.tile([16, BSUB], F32, tag="dsc")
                        nc.vector.tensor_sub(dsc, sa, ps[:16, :])
                        nc.scalar.activation(wa, dsc, AF.Sigmoid, scale=0.125)

                VaT = qkv.tile([128, EC, BSUB], BF16, tag="kv")
                VbT = qkv.tile([128, EC, BSUB], BF16, tag="kv")
                proj(w_inT_r[name], 2 * E, [XT[ai], XT[bi]], bias_in_sb[name],
                     16, [VaT, VbT], BF16)

                # -- combine: o = Vb + wa_exp * (Va - Vb)   (into VbT)
                for k in range(EC):
                    pse = at_ps.tile([128, BSUB], F32, tag="attn")
                    _mm(nc, pse, selt[:, k * 128:(k + 1) * 128], wa,
                        True, True)
                    diff = small.tile([128, BSUB], BF16, tag="diff")
                    nc.vector.tensor_sub(diff, VaT[:, k, :], VbT[:, k, :])
                    nc.vector.tensor_mul(out=diff, in0=pse, in1=diff)
                    nc.vector.tensor_add(VbT[:, k, :], diff, VbT[:, k, :])

                # -- output projection
                At = atp.tile([128, EC, BSUB], BF16, tag="at")
                proj(w_outT_r[name], 0, [VbT], bias_out_sb[name], 0,
                     [At], BF16)
                AT.append(At)

            # -- fuse: y^T = w_fuse^T-blocks @ concat(AT)
            YT = ytp.tile([128, EC, BSUB], F32, tag="yt")
            for c in range(EC):
                fb = fband.tile([128, 3 * EC, 128], BF16, tag="fb")
                nc.sync.dma_start(
                    fb, w_fuseT_r[:, :, c * 128:(c + 1) * 128])
                ps = mm_ps.tile([128, BSUB], F32, tag="mm")
                for k in range(3 * EC):
                    _mm(nc, ps, fb[:, k, :], AT[k // EC][:, k % EC, :],
                        (k == 0), (k == 3 * EC - 1))
                nc.scalar.add(YT[:, c, :], ps, bfuse_sb[:, c:c + 1])

            # -- transpose back + LayerNorm + store
            for bt in range(BSUB // 128):
                y = ypool.tile([128, E], F32, tag="y")
                for c in range(EC):
                    ps = tp_ps.tile([128, 128], F32, tag="tp")
                    nc.tensor.transpose(
                        ps, YT[:, c, bt * 128:(bt + 1) * 128], ident_f)
                    nc.scalar.copy(y[:, c * 128:(c + 1) * 128], ps)
                ssum = stats.tile([128, 1], F32, tag="ssum")
                nc.vector.reduce_sum(ssum, y, axis=AX.X)
                ysq = ysqp.tile([128, E], F32, tag="ysq")
                ss = stats.tile([128, 1], F32, tag="ss")
                nc.scalar.activation(ysq, y, AF.Square, accum_out=ss)
                mu = stats.tile([128, 1], F32, tag="mu")
                nc.vector.tensor_scalar_mul(mu, ssum, 1.0 / E)
                ex2 = stats.tile([128, 1], F32, tag="ex2")
                nc.vector.tensor_scalar_mul(ex2, ss, 1.0 / E)
                m2 = stats.tile([128, 1], F32, tag="m2")
                nc.vector.tensor_mul(out=m2, in0=mu, in1=mu)
                var = stats.tile([128, 1], F32, tag="var")
                nc.vector.tensor_sub(var, ex2, m2)
                std = stats.tile([128, 1], F32, tag="std")
                nc.scalar.activation(std, var, AF.Sqrt, bias=eps_sb)
                rstd = stats.tile([128, 1], F32, tag="rstd")
                nc.vector.reciprocal(rstd, std)
                nc.vector.tensor_scalar(y, y, mu, rstd, ALU.subtract, ALU.mult)
                nc.vector.tensor_mul(out=y, in0=y, in1=g_bc)
                nc.vector.tensor_add(y, y, b_bc)
                nc.sync.dma_start(
                    out_ap[row0 + bt * 128:row0 + (bt + 1) * 128, :], y)

        if iters == 1:
            phase_b()
        else:
            with tc.For_i(0, iters, 1) as _i:
                phase_b(_i)


_NC_CACHE = {}


def _get_program():
    if "nc" not in _NC_CACHE:
        _NC_CACHE["nc"] = build_program()
    return _NC_CACHE["nc"]


def _get_runner():
    """Cached jitted SPMD runner. Feats/outputs sharded over cores, weights
    replicated (sent once, not 8x)."""
    if "runner" in _NC_CACHE:
        return _NC_CACHE["runner"]
    import jax
    from jax.sharding import Mesh, PartitionSpec as P
    from jax.experimental.shard_map import shard_map
    from concourse import bass2jax
    from concourse.bass2jax import (_bass_exec_p, install_neuronx_cc_hook,
                                    partition_id_tensor)

    nc = _get_program()
    install_neuronx_cc_hook()
    assert nc.dbg_addr is None
    pid_name = (nc.partition_id_tensor.name
                if nc.partition_id_tensor is not None else None)

    in_names, out_names, out_avals = [], [], []
    for alloc in nc.m.functions[0].allocations:
        if not isinstance(alloc, mybir.MemoryLocationSet):
            continue
        name = alloc.memorylocations[0].name
        if alloc.kind == "ExternalInput":
            if name != pid_name:
                in_names.append(name)
        elif alloc.kind == "ExternalOutput":
            out_names.append(name)
            out_avals.append(jax.core.ShapedArray(
                tuple(alloc.tensor_shape), mybir.dt.np(alloc.dtype)))
    n_params = len(in_names)

    all_in_names = in_names + out_names + ([pid_name] if pid_name else [])

    def _body(*args):
        operands = list(args)
        if pid_name is not None:
            operands.append(partition_id_tensor())
        outs = _bass_exec_p.bind(
            *operands,
            out_avals=tuple(out_avals),
            in_names=tuple(all_in_names),
            out_names=tuple(out_names),
            lowering_input_output_aliases=(),
            sim_require_finite=True,
            sim_require_nnan=True,
            nc=nc,
        )
        return tuple(outs)

    devices = jax.devices()[:NCORES]
    mesh = Mesh(np.asarray(devices), ("core",))
    in_specs = tuple(P("core") if n in FEATS else P() for n in in_names) + \
        (P("core"),) * len(out_names)
    out_specs = (P("core"),) * len(out_names)
    sharded = jax.jit(
        shard_map(_body, mesh=mesh, in_specs=in_specs, out_specs=out_specs,
                  check_rep=False),
        donate_argnums=tuple(range(n_params, n_params + len(out_names))),
        keep_unused=True)
    _NC_CACHE["runner"] = (sharded, in_names, out_names, out_avals)
    return _NC_CACHE["runner"]


def kernel(**inputs):
    inputs = {k: np.asarray(v) for k, v in inputs.items()}
    sharded, in_names, out_names, out_avals = _get_runner()
    args = [inputs[n] for n in in_names]
    zeros = [np.zeros((NCORES * a.shape[0], *a.shape[1:]), a.dtype)
             for a in out_avals]
    outs = sharded(*args, *zeros)
    return np.asarray(outs[0])
